# revision 42
# baseline (speedup 1.0000x reference)
"""Trainium2 Bass kernel for nn_CfaModel (retrieval_knn).

Computes, for features [16, 3136, 1792], memory_bank [1792, 3136], radius [1]:
    distance[b,n,k] = ||f[b,n]||^2 + ||c[k]||^2 - 2 f.c
    vals = 6 smallest distances per (b,n)  (ascending)
    l_att = (1/NU) * mean(relu(vals[..., :3] - r^2))
    l_rep = (1/NU) * mean(relu(r^2 - vals[..., 3:] - ALPHA))
    out   = l_att + l_rep   (scalar, float32)

Strategy: data-parallel over batch across 8 NeuronCores (2 samples each).
Per core, stream 128-row tiles of features; TensorE computes
g' = 2 f.c - ||c||^2 via bf16 matmuls (f transposed on-chip via PE
transpose); VectorE max8 extracts the 8 largest g' per row (= 8 smallest
distances, since distance = ||f||^2 - g'); ScalarE accumulates
||f||^2 per row and the final relu sums.  Host sums the 8 cores'
[128, 2] partial sums and applies the 1/(NU*count) scaling.
"""

import os
import threading

import numpy as np
import ml_dtypes

import concourse.bass as bass
import concourse.mybir as mybir
import concourse.tile as tile
from concourse import bacc
import concourse.bass_utils as bass_utils
from concourse.bass_utils import run_bass_kernel_spmd
from concourse.masks import make_identity

def _dedupe_ldweights(bir_bytes: bytes) -> bytes:
    """Remove redundant standalone Ldweights from the serialized BIR.

    bass emits an Ldweights before every Matmult ("ldweights": false on the
    matmult, i.e. matmuls never self-load).  When consecutive matmuls use
    the identical stationary operand, the repeated loads are pure overhead
    (~213ns each for fp8 DoubleRow, more than the matmul itself).  Drop an
    Ldweights when its full payload matches the previous weight load on the
    same engine stream and it carries no semaphore waits/updates.
    """
    import orjson

    obj = orjson.loads(bir_bytes)
    removed = 0
    for fn in obj["functions"]:
        for blk in fn["blocks"]:
            insts = blk["instructions"]
            last_sig = None
            out = []
            for inst in insts:
                op = inst.get("opcode")
                if op == "Ldweights":
                    sync = inst.get("sync_info") or {}
                    clean = not sync.get("on_wait") and not sync.get("on_update")
                    sig = orjson.dumps(
                        {k: v for k, v in sorted(inst.items())
                         if k not in ("name", "debug", "sync_info")}
                    )
                    if clean and sig == last_sig:
                        removed += 1
                        continue
                    # a waited load still reloads, but it's a safe barrier;
                    # afterwards the loaded weights match sig either way
                    last_sig = sig
                elif op == "Matmult":
                    pass  # never self-loads; leaves weights untouched
                elif inst.get("engine") == "PE" and op not in (
                    "EventSemaphore", "Drain", "Nop"
                ):
                    last_sig = None
                out.append(inst)
            blk["instructions"] = out
    return orjson.dumps(obj)

# Problem constants (hardcoded per the harness contract).
B, HW, C, K = 16, 3136, 1792, 3136
NU, ALPHA = 0.001, 0.1
NCORES = 8
BPC = B // NCORES          # batches per core = 2
ROWS = BPC * HW            # rows per core = 6272
P = 128                    # partitions
NT = ROWS // P             # row tiles per core = 49
KC = C // P                # contraction chunks = 14
CT = 7                     # column tiles
CW = K // CT               # column tile width = 448

FP32 = mybir.dt.float32
BF16 = mybir.dt.bfloat16
FP8 = mybir.dt.float8e4
AF = mybir.ActivationFunctionType
USE_FP8 = bool(int(os.environ.get("KNN_FP8", "1")))


def build_module(nt=NT, fp8=USE_FP8):
    nc = bacc.Bacc(trn_type="TRN2", target_bir_lowering=False)
    mm_dt = FP8 if fp8 else BF16

    f_dram = nc.dram_tensor("f", [nt, P, C], BF16, kind="ExternalInput")
    m2_dram = nc.dram_tensor("m2", [P, KC, K], mm_dt, kind="ExternalInput")
    r_dram = nc.dram_tensor("radius", [1, 1], FP32, kind="ExternalInput")
    out_dram = nc.dram_tensor("out", [P, 2], FP32, kind="ExternalOutput")

    with tile.TileContext(nc) as tc:
        with tc.tile_pool(name="singles", bufs=1) as singles:
            # ---- persistent tiles ----
            ident = singles.tile([P, P], BF16)
            make_identity(nc, ident[:])
            ones_bf = singles.tile([P, P], BF16)
            # lhsT for the c_sq reduction: msq holds (2m)^2, so sum with
            # weight 1/4 to recover sum(m^2)
            nc.vector.memset(ones_bf[:], 0.25)
            c_sqb = singles.tile([P, K], FP32)           # ||c||^2 broadcast on partitions
            g_all = singles.tile([P, nt, 8], FP32)       # top-8 of g' per row tile
            fsq_all = singles.tile([P, nt], FP32)        # ||f||^2 per row

            # radius -> biases: neg_r2 = -r^2, r2ma = r^2 - ALPHA
            rb = singles.tile([P, 1], FP32)
            nc.gpsimd.dma_start(out=rb[:], in_=r_dram[:].to_broadcast([P, 1]))
            neg_r2 = singles.tile([P, 1], FP32)
            r2ma = singles.tile([P, 1], FP32)
            nc.vector.tensor_mul(neg_r2[:], rb[:], rb[:])
            nc.vector.tensor_scalar_add(r2ma[:], neg_r2[:], -ALPHA)
            nc.vector.tensor_scalar_mul(neg_r2[:], neg_r2[:], -1.0)

            m2 = singles.tile([P, KC, K], mm_dt)         # 2*memory_bank, C on partitions

            # ---- main pools (also reused by the c_sq setup) ----
            with (
                tc.tile_pool(name="fp", bufs=3) as fp,
                tc.tile_pool(name="ftp", bufs=2) as ftp,
                tc.tile_pool(name="sqp", bufs=2) as sqp,
                tc.tile_pool(name="gpp", bufs=1 if fp8 else 2) as gpp,
                tc.tile_pool(name="mmp", bufs=4, space="PSUM") as mmp,
                tc.tile_pool(name="tpp", bufs=2, space="PSUM") as tpp,
            ):
                def load_tile(t):
                    f_t = fp.tile([P, C], BF16, name="f_t")
                    nc.sync.dma_start(f_t[:], f_dram[t])
                    return f_t

                def emit_square(f_t, t):
                    # ||f||^2 per row (sum of squares along free axis).
                    # Emitted after the transpose copies so it doesn't delay
                    # them in the ScalarE queue.
                    sq = sqp.tile([P, C], BF16, name="sq")
                    nc.scalar.activation(
                        sq[:], f_t[:], AF.Square,
                        accum_out=fsq_all[:, t:t + 1],
                    )

                def emit_transposes(f_t, fT, half):
                    # transpose f tile (bf16 via PE), cast to matmul dtype
                    # during the PSUM->SBUF copy; 8 (then 6) transposes are
                    # packed into one PSUM bank so ScalarE does 2 copies
                    # per row tile
                    lo, hi = (0, 8) if half == 0 else (8, KC)
                    ptp = tpp.tile([P, 8, P], BF16, name="ptp")
                    last = None
                    for idx, ci in enumerate(range(lo, hi)):
                        last = nc.tensor.transpose(
                            ptp[:, idx, :],
                            f_t[:, ci * P:(ci + 1) * P],
                            ident[:],
                        )
                    nc.scalar.copy(fT[:, lo:hi, :], ptp[:, :hi - lo, :])
                    return last

                NQ = KC // 2
                # f(0) first so its DMA isn't queued behind the m2 load
                cur_f = load_tile(0)
                cur_fT = ftp.tile([P, KC, P], mm_dt, name="fT")
                emit_transposes(cur_f, cur_fT, 0)
                emit_transposes(cur_f, cur_fT, 1)
                emit_square(cur_f, 0)

                # ---- m2 load + squares for c_sq ----
                # per-chunk DMA (alternating two DMA paths) so the squares
                # pipeline with the load; squares alternate ScalarE/VectorE
                # (each alone is slower than the DMA cadence).  The actual
                # ones^T @ msq reduction is interleaved into tile 0's matmul
                # stream below so it runs on a warm PE and overlaps the
                # remaining setup instead of serializing before the loop.
                msq = singles.tile([P, KC, K], BF16)
                H = K // 2
                for ci in range(KC):
                    # both halves in flight on separate DMA paths
                    nc.sync.dma_start(m2[:, ci, :H], m2_dram[:, ci, :H])
                    nc.gpsimd.dma_start(m2[:, ci, H:], m2_dram[:, ci, H:])
                    if ci % 2 == 0:
                        nc.scalar.activation(
                            msq[:, ci, :], m2[:, ci, :], AF.Square
                        )
                    else:
                        nc.vector.tensor_mul(
                            msq[:, ci, :], m2[:, ci, :], m2[:, ci, :]
                        )

                for j in range(CT):
                    cs = mmp.tile([P, CW], FP32, name="acc")
                    for ci in range(KC):
                        nc.tensor.matmul(
                            cs[:],
                            ones_bf[:],
                            msq[:, ci, j * CW:(j + 1) * CW],
                            start=(ci == 0),
                            stop=(ci == KC - 1),
                        )
                    nc.scalar.copy(c_sqb[:, j * CW:(j + 1) * CW], cs[:])

                for t in range(nt):
                    nxt_f = nxt_fT = None
                    if t + 1 < nt:
                        nxt_f = load_tile(t + 1)
                        nxt_fT = ftp.tile([P, KC, P], mm_dt, name="fT")

                    # g' = 2 f.c - c_sq   (columns tiled by CW)
                    gp = gpp.tile([P, K], FP32)
                    if fp8:
                        for j in range(CT):
                            # next tile's transposes interleave into this
                            # tile's matmul stream so their ScalarE copies
                            # overlap matmuls instead of stalling the PE at
                            # the row-tile boundary
                            if nxt_f is not None and j in (1, 4):
                                emit_transposes(nxt_f, nxt_fT, 0 if j == 1 else 1)
                            if nxt_f is not None and j == 5:
                                emit_square(nxt_f, t + 1)
                            mm = mmp.tile([P, CW], FP32, name="acc")
                            for q in range(NQ):
                                nc.tensor.matmul(
                                    mm[:],
                                    cur_fT[:, 2 * q:2 * q + 2, :],
                                    m2[:, 2 * q:2 * q + 2, j * CW:(j + 1) * CW],
                                    start=(q == 0),
                                    stop=(q == NQ - 1),
                                    perf_mode=mybir.MatmulPerfMode.DoubleRow,
                                )
                            nc.vector.tensor_sub(
                                gp[:, j * CW:(j + 1) * CW], mm[:],
                                c_sqb[:, j * CW:(j + 1) * CW],
                            )
                    else:
                        if nxt_f is not None:
                            emit_transposes(nxt_f, nxt_fT, 0)
                            emit_transposes(nxt_f, nxt_fT, 1)
                            emit_square(nxt_f, t + 1)
                        for j in range(CT):
                            mm = mmp.tile([P, CW], FP32)
                            for ci in range(KC):
                                nc.tensor.matmul(
                                    mm[:],
                                    cur_fT[:, ci, :],
                                    m2[:, ci, j * CW:(j + 1) * CW],
                                    start=(ci == 0),
                                    stop=(ci == KC - 1),
                                )
                            nc.vector.tensor_sub(
                                gp[:, j * CW:(j + 1) * CW], mm[:],
                                c_sqb[:, j * CW:(j + 1) * CW],
                            )

                    # top-8 largest g' (descending) == 8 smallest distances
                    nc.vector.max(out=g_all[:, t, :], in_=gp[:])
                    cur_f, cur_fT = nxt_f, nxt_fT

            # ---- epilogue: loss partial sums ----
            u_all = singles.tile([P, nt, 8], FP32)       # u = g' - ||f||^2 = -distance
            nc.vector.tensor_sub(
                u_all[:], g_all[:],
                fsq_all[:, :, None].to_broadcast([P, nt, 8]),
            )
            att_scr = singles.tile([P, nt, 3], FP32)
            rep_scr = singles.tile([P, nt, 3], FP32)
            outp = singles.tile([P, 2], FP32)
            # att = relu(distance - r^2) = relu(-u - r^2)
            nc.scalar.activation(
                att_scr[:], u_all[:, :, 0:3], AF.Relu,
                bias=neg_r2[:], scale=-1.0,
                accum_out=outp[:, 0:1],
            )
            # rep = relu(r^2 - distance - ALPHA) = relu(u + (r^2 - ALPHA))
            nc.scalar.activation(
                rep_scr[:], u_all[:, :, 3:6], AF.Relu,
                bias=r2ma[:], scale=1.0,
                accum_out=outp[:, 1:2],
            )
            nc.sync.dma_start(out_dram[:], outp[:])

    nc.compile()
    if bool(int(os.environ.get("KNN_LDWDEDUP", "1"))):
        orig_to_json = nc.to_json_bytes
        nc.to_json_bytes = lambda: _dedupe_ldweights(orig_to_json())
    return nc


_CACHE = {}
_LOCK = threading.Lock()
LAST_RESULT = None


def _get_module(nt=NT, fp8=USE_FP8):
    with _LOCK:
        if (nt, fp8) not in _CACHE:
            _CACHE[(nt, fp8)] = build_module(nt, fp8)
        return _CACHE[(nt, fp8)]


def prep_inputs(features, memory_bank, radius, fp8=USE_FP8):
    mm_np = ml_dtypes.float8_e4m3 if fp8 else ml_dtypes.bfloat16
    f_bf = features.reshape(NCORES, NT, P, C).astype(ml_dtypes.bfloat16)
    m2 = (
        (2.0 * memory_bank)
        .reshape(KC, P, K)
        .transpose(1, 0, 2)
        .astype(mm_np)
        .copy()
    )
    r2d = radius.reshape(1, 1).astype(np.float32)
    return f_bf, m2, r2d


def kernel(features, memory_bank, radius):
    global LAST_RESULT
    features = np.asarray(features, dtype=np.float32)
    memory_bank = np.asarray(memory_bank, dtype=np.float32)
    radius = np.asarray(radius, dtype=np.float32)
    assert features.shape == (B, HW, C)
    assert memory_bank.shape == (C, K)

    nc = _get_module()

    # Shard: batch-parallel, 2 samples per core.  Low-precision cast on
    # host; the f^2 / c^2 / top-k arithmetic stays fp32 on device.
    f_bf, m2, r2d = prep_inputs(features, memory_bank, radius)

    in_maps = [{"f": f_bf[i], "m2": m2, "radius": r2d} for i in range(NCORES)]
    trace = bool(int(os.environ.get("KNN_TRACE", "0")))
    try:
        res = run_bass_kernel_spmd(
            nc, in_maps, core_ids=list(range(NCORES)), trace=trace
        )
    except ModuleNotFoundError:
        # axon NTFF profiling hook unavailable in this environment
        res = run_bass_kernel_spmd(
            nc, in_maps, core_ids=list(range(NCORES)), trace=False
        )
    LAST_RESULT = res

    parts = np.stack([r["out"] for r in res.results])   # [8, 128, 2]
    total = parts.sum(axis=(0, 1), dtype=np.float64)    # [sum_att, sum_rep]
    cnt = B * HW * 3
    loss = (total[0] + total[1]) / cnt / NU
    return np.float32(loss)


# revision 47
# speedup vs baseline: 1.0729x; 1.0729x over previous
"""Trainium2 Bass kernel for nn_CfaModel (retrieval_knn).

Computes, for features [16, 3136, 1792], memory_bank [1792, 3136], radius [1]:
    distance[b,n,k] = ||f[b,n]||^2 + ||c[k]||^2 - 2 f.c
    vals = 6 smallest distances per (b,n)  (ascending)
    l_att = (1/NU) * mean(relu(vals[..., :3] - r^2))
    l_rep = (1/NU) * mean(relu(r^2 - vals[..., 3:] - ALPHA))
    out   = l_att + l_rep   (scalar, float32)

Strategy: data-parallel over batch across 8 NeuronCores (2 samples each).
Per core, stream 128-row tiles of features; TensorE computes
g' = 2 f.c - ||c||^2 via bf16 matmuls (f transposed on-chip via PE
transpose); VectorE max8 extracts the 8 largest g' per row (= 8 smallest
distances, since distance = ||f||^2 - g'); ScalarE accumulates
||f||^2 per row and the final relu sums.  Host sums the 8 cores'
[128, 2] partial sums and applies the 1/(NU*count) scaling.
"""

import os
import threading

import numpy as np
import ml_dtypes

import concourse.bass as bass
import concourse.mybir as mybir
import concourse.tile as tile
from concourse import bacc
import concourse.bass_utils as bass_utils
from concourse.bass_utils import run_bass_kernel_spmd
from concourse.masks import make_identity

def _dedupe_ldweights(bir_bytes: bytes) -> bytes:
    """Remove redundant standalone Ldweights from the serialized BIR.

    bass emits an Ldweights before every Matmult ("ldweights": false on the
    matmult, i.e. matmuls never self-load).  When consecutive matmuls use
    the identical stationary operand, the repeated loads are pure overhead
    (~213ns each for fp8 DoubleRow, more than the matmul itself).  Drop an
    Ldweights when its full payload matches the previous weight load on the
    same engine stream and it carries no semaphore waits/updates.
    """
    import orjson

    obj = orjson.loads(bir_bytes)
    removed = 0
    for fn in obj["functions"]:
        for blk in fn["blocks"]:
            insts = blk["instructions"]
            last_sig = None
            out = []
            for inst in insts:
                op = inst.get("opcode")
                if op == "Ldweights":
                    sync = inst.get("sync_info") or {}
                    clean = not sync.get("on_wait") and not sync.get("on_update")
                    sig = orjson.dumps(
                        {k: v for k, v in sorted(inst.items())
                         if k not in ("name", "debug", "sync_info")}
                    )
                    if clean and sig == last_sig:
                        removed += 1
                        continue
                    # a waited load still reloads, but it's a safe barrier;
                    # afterwards the loaded weights match sig either way
                    last_sig = sig
                elif op == "Matmult":
                    pass  # never self-loads; leaves weights untouched
                elif inst.get("engine") == "PE" and op not in (
                    "EventSemaphore", "Drain", "Nop"
                ):
                    last_sig = None
                out.append(inst)
            blk["instructions"] = out
    return orjson.dumps(obj)

# Problem constants (hardcoded per the harness contract).
B, HW, C, K = 16, 3136, 1792, 3136
NU, ALPHA = 0.001, 0.1
NCORES = 8
BPC = B // NCORES          # batches per core = 2
ROWS = BPC * HW            # rows per core = 6272
P = 128                    # partitions
NT = ROWS // P             # row tiles per core = 49
KC = C // P                # contraction chunks = 14
CT = 7                     # column tiles
CW = K // CT               # column tile width = 448

FP32 = mybir.dt.float32
BF16 = mybir.dt.bfloat16
FP8 = mybir.dt.float8e4
AF = mybir.ActivationFunctionType
USE_FP8 = bool(int(os.environ.get("KNN_FP8", "1")))


def build_module(nt=NT, fp8=USE_FP8):
    nc = bacc.Bacc(trn_type="TRN2", target_bir_lowering=False)
    mm_dt = FP8 if fp8 else BF16

    f_dram = nc.dram_tensor("f", [nt, P, C], BF16, kind="ExternalInput")
    # pre-transposed f (c on partitions) for the matmul stationary operand;
    # avoids 14 PE transposes + 2 ScalarE copies per row tile
    fT_dram = nc.dram_tensor("fT", [nt, P, KC, P], mm_dt, kind="ExternalInput")
    m2_dram = nc.dram_tensor("m2", [P, KC, K], mm_dt, kind="ExternalInput")
    r_dram = nc.dram_tensor("radius", [1, 1], FP32, kind="ExternalInput")
    out_dram = nc.dram_tensor("out", [P, 2], FP32, kind="ExternalOutput")

    with tile.TileContext(nc) as tc:
        with tc.tile_pool(name="singles", bufs=1) as singles:
            # ---- persistent tiles ----
            ident = singles.tile([P, P], BF16)
            make_identity(nc, ident[:])
            ones_bf = singles.tile([P, P], BF16)
            # lhsT for the c_sq reduction: msq holds (2m)^2, so sum with
            # weight 1/4 to recover sum(m^2)
            nc.vector.memset(ones_bf[:], 0.25)
            c_sqb = singles.tile([P, K], FP32)           # ||c||^2 broadcast on partitions
            g_all = singles.tile([P, nt, 8], FP32)       # top-8 of g' per row tile
            fsq_all = singles.tile([P, nt], FP32)        # ||f||^2 per row

            # radius -> biases: neg_r2 = -r^2, r2ma = r^2 - ALPHA
            rb = singles.tile([P, 1], FP32)
            nc.gpsimd.dma_start(out=rb[:], in_=r_dram[:].to_broadcast([P, 1]))
            neg_r2 = singles.tile([P, 1], FP32)
            r2ma = singles.tile([P, 1], FP32)
            nc.vector.tensor_mul(neg_r2[:], rb[:], rb[:])
            nc.vector.tensor_scalar_add(r2ma[:], neg_r2[:], -ALPHA)
            nc.vector.tensor_scalar_mul(neg_r2[:], neg_r2[:], -1.0)

            m2 = singles.tile([P, KC, K], mm_dt)         # 2*memory_bank, C on partitions

            # ---- main pools (also reused by the c_sq setup) ----
            with (
                tc.tile_pool(name="fp", bufs=3) as fp,
                tc.tile_pool(name="ftp", bufs=2) as ftp,
                tc.tile_pool(name="sqp", bufs=2) as sqp,
                tc.tile_pool(name="gpp", bufs=1 if fp8 else 2) as gpp,
                tc.tile_pool(name="mmp", bufs=4, space="PSUM") as mmp,
                tc.tile_pool(name="tpp", bufs=2, space="PSUM") as tpp,
            ):
                def load_tile(t):
                    f_t = fp.tile([P, C], BF16, name="f_t")
                    nc.sync.dma_start(f_t[:], f_dram[t])
                    return f_t

                def emit_square(f_t, t):
                    # ||f||^2 per row (sum of squares along free axis).
                    # Emitted after the transpose copies so it doesn't delay
                    # them in the ScalarE queue.
                    sq = sqp.tile([P, C], BF16, name="sq")
                    nc.scalar.activation(
                        sq[:], f_t[:], AF.Square,
                        accum_out=fsq_all[:, t:t + 1],
                    )

                def emit_transposes(f_t, fT, half):
                    # transpose f tile (bf16 via PE), cast to matmul dtype
                    # during the PSUM->SBUF copy; 8 (then 6) transposes are
                    # packed into one PSUM bank so ScalarE does 2 copies
                    # per row tile
                    lo, hi = (0, 8) if half == 0 else (8, KC)
                    ptp = tpp.tile([P, 8, P], BF16, name="ptp")
                    last = None
                    for idx, ci in enumerate(range(lo, hi)):
                        last = nc.tensor.transpose(
                            ptp[:, idx, :],
                            f_t[:, ci * P:(ci + 1) * P],
                            ident[:],
                        )
                    nc.scalar.copy(fT[:, lo:hi, :], ptp[:, :hi - lo, :])
                    return last

                def load_ft(t):
                    fT_t = ftp.tile([P, KC, P], mm_dt, name="fT")
                    nc.gpsimd.dma_start(fT_t[:], fT_dram[t])
                    return fT_t

                NQ = KC // 2
                # f(0)/fT(0) first so their DMAs aren't queued behind m2
                cur_f = load_tile(0)
                cur_fT = load_ft(0)
                emit_square(cur_f, 0)

                # ---- m2 load + squares for c_sq ----
                # per-chunk DMA (alternating two DMA paths) so the squares
                # pipeline with the load; squares alternate ScalarE/VectorE
                # (each alone is slower than the DMA cadence).  The actual
                # ones^T @ msq reduction is interleaved into tile 0's matmul
                # stream below so it runs on a warm PE and overlaps the
                # remaining setup instead of serializing before the loop.
                msq = singles.tile([P, KC, K], BF16)
                H = K // 2
                for ci in range(KC):
                    # both halves in flight on separate DMA paths
                    nc.sync.dma_start(m2[:, ci, :H], m2_dram[:, ci, :H])
                    nc.gpsimd.dma_start(m2[:, ci, H:], m2_dram[:, ci, H:])
                    if ci % 2 == 0:
                        nc.scalar.activation(
                            msq[:, ci, :], m2[:, ci, :], AF.Square
                        )
                    else:
                        nc.vector.tensor_mul(
                            msq[:, ci, :], m2[:, ci, :], m2[:, ci, :]
                        )

                for j in range(CT):
                    cs = mmp.tile([P, CW], FP32, name="acc")
                    for ci in range(KC):
                        nc.tensor.matmul(
                            cs[:],
                            ones_bf[:],
                            msq[:, ci, j * CW:(j + 1) * CW],
                            start=(ci == 0),
                            stop=(ci == KC - 1),
                        )
                    nc.scalar.copy(c_sqb[:, j * CW:(j + 1) * CW], cs[:])

                for t in range(nt):
                    nxt_f = nxt_fT = None
                    if t + 1 < nt:
                        nxt_f = load_tile(t + 1)
                        nxt_fT = load_ft(t + 1) if fp8 else ftp.tile(
                            [P, KC, P], mm_dt, name="fT"
                        )

                    # g' = 2 f.c - c_sq   (columns tiled by CW)
                    gp = gpp.tile([P, K], FP32)
                    if fp8:
                        for j in range(CT):
                            if nxt_f is not None and j == 5:
                                emit_square(nxt_f, t + 1)
                            mm = mmp.tile([P, CW], FP32, name="acc")
                            for q in range(NQ):
                                nc.tensor.matmul(
                                    mm[:],
                                    cur_fT[:, 2 * q:2 * q + 2, :],
                                    m2[:, 2 * q:2 * q + 2, j * CW:(j + 1) * CW],
                                    start=(q == 0),
                                    stop=(q == NQ - 1),
                                    perf_mode=mybir.MatmulPerfMode.DoubleRow,
                                )
                            nc.vector.tensor_sub(
                                gp[:, j * CW:(j + 1) * CW], mm[:],
                                c_sqb[:, j * CW:(j + 1) * CW],
                            )
                    else:
                        if nxt_f is not None:
                            emit_transposes(nxt_f, nxt_fT, 0)
                            emit_transposes(nxt_f, nxt_fT, 1)
                            emit_square(nxt_f, t + 1)
                        for j in range(CT):
                            mm = mmp.tile([P, CW], FP32)
                            for ci in range(KC):
                                nc.tensor.matmul(
                                    mm[:],
                                    cur_fT[:, ci, :],
                                    m2[:, ci, j * CW:(j + 1) * CW],
                                    start=(ci == 0),
                                    stop=(ci == KC - 1),
                                )
                            nc.vector.tensor_sub(
                                gp[:, j * CW:(j + 1) * CW], mm[:],
                                c_sqb[:, j * CW:(j + 1) * CW],
                            )

                    # top-8 largest g' (descending) == 8 smallest distances
                    nc.vector.max(out=g_all[:, t, :], in_=gp[:])
                    cur_f, cur_fT = nxt_f, nxt_fT

            # ---- epilogue: loss partial sums ----
            u_all = singles.tile([P, nt, 8], FP32)       # u = g' - ||f||^2 = -distance
            nc.vector.tensor_sub(
                u_all[:], g_all[:],
                fsq_all[:, :, None].to_broadcast([P, nt, 8]),
            )
            att_scr = singles.tile([P, nt, 3], FP32)
            rep_scr = singles.tile([P, nt, 3], FP32)
            outp = singles.tile([P, 2], FP32)
            # att = relu(distance - r^2) = relu(-u - r^2)
            nc.scalar.activation(
                att_scr[:], u_all[:, :, 0:3], AF.Relu,
                bias=neg_r2[:], scale=-1.0,
                accum_out=outp[:, 0:1],
            )
            # rep = relu(r^2 - distance - ALPHA) = relu(u + (r^2 - ALPHA))
            nc.scalar.activation(
                rep_scr[:], u_all[:, :, 3:6], AF.Relu,
                bias=r2ma[:], scale=1.0,
                accum_out=outp[:, 1:2],
            )
            nc.sync.dma_start(out_dram[:], outp[:])

    nc.compile()
    if bool(int(os.environ.get("KNN_LDWDEDUP", "1"))):
        orig_to_json = nc.to_json_bytes
        nc.to_json_bytes = lambda: _dedupe_ldweights(orig_to_json())
    return nc


_CACHE = {}
_LOCK = threading.Lock()
LAST_RESULT = None


def _get_module(nt=NT, fp8=USE_FP8):
    with _LOCK:
        if (nt, fp8) not in _CACHE:
            _CACHE[(nt, fp8)] = build_module(nt, fp8)
        return _CACHE[(nt, fp8)]


def prep_inputs(features, memory_bank, radius, fp8=USE_FP8):
    mm_np = ml_dtypes.float8_e4m3 if fp8 else ml_dtypes.bfloat16
    f_bf = features.reshape(NCORES, NT, P, C).astype(ml_dtypes.bfloat16)
    # pre-transposed matmul operand: [core, t, p(=c%128), ci, r]
    fT = np.ascontiguousarray(
        f_bf.reshape(NCORES, NT, P, KC, P).transpose(0, 1, 4, 3, 2)
    ).astype(mm_np)
    m2 = (
        (2.0 * memory_bank)
        .reshape(KC, P, K)
        .transpose(1, 0, 2)
        .astype(mm_np)
        .copy()
    )
    r2d = radius.reshape(1, 1).astype(np.float32)
    return f_bf, fT, m2, r2d


def kernel(features, memory_bank, radius):
    global LAST_RESULT
    features = np.asarray(features, dtype=np.float32)
    memory_bank = np.asarray(memory_bank, dtype=np.float32)
    radius = np.asarray(radius, dtype=np.float32)
    assert features.shape == (B, HW, C)
    assert memory_bank.shape == (C, K)

    nc = _get_module()

    # Shard: batch-parallel, 2 samples per core.  Low-precision cast on
    # host; the f^2 / c^2 / top-k arithmetic stays fp32 on device.
    f_bf, fT, m2, r2d = prep_inputs(features, memory_bank, radius)

    in_maps = [
        {"f": f_bf[i], "fT": fT[i], "m2": m2, "radius": r2d}
        for i in range(NCORES)
    ]
    trace = bool(int(os.environ.get("KNN_TRACE", "0")))
    try:
        res = run_bass_kernel_spmd(
            nc, in_maps, core_ids=list(range(NCORES)), trace=trace
        )
    except ModuleNotFoundError:
        # axon NTFF profiling hook unavailable in this environment
        res = run_bass_kernel_spmd(
            nc, in_maps, core_ids=list(range(NCORES)), trace=False
        )
    LAST_RESULT = res

    parts = np.stack([r["out"] for r in res.results])   # [8, 128, 2]
    total = parts.sum(axis=(0, 1), dtype=np.float64)    # [sum_att, sum_rep]
    cnt = B * HW * 3
    loss = (total[0] + total[1]) / cnt / NU
    return np.float32(loss)
